# revision 1
# baseline (speedup 1.0000x reference)
"""Trainium2 kernel for nn_ConvNN_2D_Spatial_K_N_Location.

Strategy (8 NeuronCores, pure data parallel over batch):
  - The two KNN-conv layers (irregular top-9 selection/gather, ~6% of FLOPs)
    are computed on host in fp32 with reference-exact tie-breaking.
  - The dominant FC stack (fc1: 1024x32768x1024 = 68.7 GFLOP + fc2) runs on
    the 8 NeuronCores: batch sharded 128 rows/core, fw1 replicated and
    streamed HBM->SBUF in 512KB chunks, fp32 matmuls accumulating in PSUM.
"""
import numpy as np

import concourse.bass as bass
import concourse.tile as tile
from concourse import bacc, mybir
from concourse.bass_utils import run_bass_kernel_spmd

K, N, SCALE = 9, 8, 2
NCORES = 8
B_LOCAL = 128          # 1024 / 8
F = 32768              # fc1 contraction
U = 1024               # fc1 output
O2 = 10                # final outputs
FCH = 256              # number of 128-row feature chunks (32768/128)

_CACHE = {}


# ---------------------------------------------------------------- host conv
def _unshuffle(x, s):
    B, C, H, W = x.shape
    return x.reshape(B, C, H//s, s, W//s, s).transpose(0, 1, 3, 5, 2, 4).reshape(B, C*s*s, H//s, W//s)


def _shuffle(x, s):
    B, C, H, W = x.shape
    return x.reshape(B, C//(s*s), s, s, H, W).transpose(0, 1, 4, 2, 5, 3).reshape(B, C//(s*s), H*s, W*s)


def _conv_nn(x, w, b):
    x = _unshuffle(x, SCALE)
    B, C, H, W = x.shape
    gy, gx = np.meshgrid(np.linspace(0., 1., H, dtype=np.float32),
                         np.linspace(0., 1., W, dtype=np.float32), indexing='ij')
    loc = np.broadcast_to(np.stack([gy, gx])[None], (B, 2, H, W)).astype(np.float32)
    x = np.concatenate([x, loc], 1)
    Cf = C + 2
    xf = x.reshape(B, Cf, H*W)
    ih = np.linspace(0, H-1, N).astype(np.int32)
    iw = np.linspace(0, W-1, N).astype(np.int32)
    samp = x[:, :, ih][:, :, :, iw].reshape(B, Cf, N*N)
    d2 = (np.einsum('bct,bct->bt', xf, xf)[:, :, None]
          - 2.0 * np.einsum('bct,bcn->btn', xf, samp, optimize=True)
          + np.einsum('bcn,bcn->bn', samp, samp)[:, None, :]).astype(np.float32)
    # top-K nearest, ties broken toward lower candidate index (== jax top_k)
    idx = np.argsort(d2, axis=2, kind='stable')[:, :, :K]
    neigh = np.take_along_axis(samp[:, :, None, :], idx[:, None, :, :], axis=3)
    ng = neigh.transpose(0, 2, 1, 3).reshape(B, H*W, Cf*K)
    out = ng @ w.reshape(w.shape[0], Cf*K).T.astype(np.float32) + b[None, None, :]
    out = out.transpose(0, 2, 1)
    return _shuffle(out.reshape(B, w.shape[0], H, W), SCALE)


# ---------------------------------------------------------------- device fc
def _build_fc_kernel():
    if 'nc' in _CACHE:
        return _CACHE['nc']
    nc = bacc.Bacc("TRN2", target_bir_lowering=False, debug=False,
                   enable_asserts=False, num_devices=NCORES)
    f32 = mybir.dt.float32
    h2t = nc.dram_tensor("h2t", (F, B_LOCAL), f32, kind="ExternalInput").ap()
    fw1t = nc.dram_tensor("fw1t", (F, U), f32, kind="ExternalInput").ap()
    fb1r = nc.dram_tensor("fb1r", (1, U), f32, kind="ExternalInput").ap()
    fw2t = nc.dram_tensor("fw2t", (U, O2), f32, kind="ExternalInput").ap()
    fb2r = nc.dram_tensor("fb2r", (1, O2), f32, kind="ExternalInput").ap()
    onesr = nc.dram_tensor("onesr", (1, B_LOCAL), f32, kind="ExternalInput").ap()
    ident = nc.dram_tensor("ident", (128, 128), f32, kind="ExternalInput").ap()
    outt = nc.dram_tensor("outt", (O2, B_LOCAL), f32, kind="ExternalOutput").ap()

    with tile.TileContext(nc) as tc:
        with tc.tile_pool(name="w", bufs=4) as wpool, \
             tc.tile_pool(name="h", bufs=4) as hpool, \
             tc.tile_pool(name="small", bufs=1) as spool, \
             tc.tile_pool(name="acts", bufs=1) as apool, \
             tc.tile_pool(name="ps", bufs=1, space="PSUM") as pspool, \
             tc.tile_pool(name="pst", bufs=2, space="PSUM") as ptpool:

            ones_t = spool.tile([1, B_LOCAL], f32)
            nc.sync.dma_start(ones_t[:], onesr[:, :])
            fb1_t = spool.tile([1, U], f32)
            nc.sync.dma_start(fb1_t[:], fb1r[:, :])
            fb2_t = spool.tile([1, O2], f32)
            nc.sync.dma_start(fb2_t[:], fb2r[:, :])
            id_t = spool.tile([128, 128], f32)
            nc.sync.dma_start(id_t[:], ident[:, :])
            fw2_t = spool.tile([128, 8 * O2], f32)
            for c in range(8):
                nc.sync.dma_start(fw2_t[:, bass.ts(c, O2)],
                                  fw2t[bass.ts(c, 128), :])

            psum1 = pspool.tile([128, U], f32)
            # fc1: accumulate over 256 feature chunks of 128
            for i in range(FCH):
                wt = wpool.tile([128, U], f32)
                nc.sync.dma_start(wt[:], fw1t[bass.ts(i, 128), :])
                ht = hpool.tile([128, B_LOCAL], f32)
                nc.sync.dma_start(ht[:], h2t[bass.ts(i, 128), :])
                for half in range(2):
                    nc.tensor.matmul(psum1[:, bass.ts(half, 512)],
                                     lhsT=ht[:],
                                     rhs=wt[:, bass.ts(half, 512)],
                                     start=(i == 0), stop=False)
            # + fb1 (outer product with ones row), closes the accumulation
            for half in range(2):
                nc.tensor.matmul(psum1[:, bass.ts(half, 512)],
                                 lhsT=ones_t[:],
                                 rhs=fb1_t[:, bass.ts(half, 512)],
                                 start=False, stop=True)

            # relu -> SBUF
            h1_t = apool.tile([128, U], f32)
            nc.scalar.activation(h1_t[:], psum1[:],
                                 mybir.ActivationFunctionType.Relu)

            # transpose h1 in 128x128 blocks (PE), then fc2
            h1T = apool.tile([128, U], f32)
            for c in range(8):
                pt = ptpool.tile([128, 128], f32)
                nc.tensor.transpose(pt[:], h1_t[:, bass.ts(c, 128)], id_t[:])
                nc.scalar.copy(h1T[:, bass.ts(c, 128)], pt[:])

            psum2 = ptpool.tile([O2, B_LOCAL], f32)
            for c in range(8):
                nc.tensor.matmul(psum2[:], lhsT=fw2_t[:, bass.ts(c, O2)],
                                 rhs=h1T[:, bass.ts(c, 128)],
                                 start=(c == 0), stop=False)
            nc.tensor.matmul(psum2[:], lhsT=fb2_t[:], rhs=ones_t[:],
                             start=False, stop=True)

            out_t = apool.tile([O2, B_LOCAL], f32)
            nc.scalar.copy(out_t[:], psum2[:])
            nc.sync.dma_start(outt[:, :], out_t[:])

    nc.compile()
    _CACHE['nc'] = nc
    return nc


def kernel(x, w1, b1, w2, b2, fw1, fb1, fw2, fb2):
    x = np.asarray(x, np.float32)
    # host: the two KNN-conv layers (exact fp32 ranking, reference tie-break)
    h1 = np.maximum(_conv_nn(x, np.asarray(w1, np.float32), np.asarray(b1, np.float32)), 0)
    h2 = np.maximum(_conv_nn(h1, np.asarray(w2, np.float32), np.asarray(b2, np.float32)), 0)
    h2 = h2.reshape(h2.shape[0], -1)                    # (1024, 32768)

    nc = _build_fc_kernel()
    fw1t = np.ascontiguousarray(np.asarray(fw1, np.float32).T)      # (32768, 1024)
    fw2t = np.ascontiguousarray(np.asarray(fw2, np.float32).T)      # (1024, 10)
    fb1r = np.asarray(fb1, np.float32).reshape(1, U)
    fb2r = np.asarray(fb2, np.float32).reshape(1, O2)
    onesr = np.ones((1, B_LOCAL), np.float32)
    ident = np.eye(128, dtype=np.float32)

    in_maps = []
    for i in range(NCORES):
        h2t = np.ascontiguousarray(h2[i*B_LOCAL:(i+1)*B_LOCAL].T)   # (32768, 128)
        in_maps.append(dict(h2t=h2t, fw1t=fw1t, fb1r=fb1r, fw2t=fw2t,
                            fb2r=fb2r, onesr=onesr, ident=ident))

    res = run_bass_kernel_spmd(nc, in_maps, core_ids=list(range(NCORES)))
    out = np.empty((NCORES * B_LOCAL, O2), np.float32)
    for i in range(NCORES):
        out[i*B_LOCAL:(i+1)*B_LOCAL] = res.results[i]["outt"].T
    return out



# revision 2
# speedup vs baseline: 2.7169x; 2.7169x over previous
"""Trainium2 kernel for nn_ConvNN_2D_Spatial_K_N_Location.

Strategy (8 NeuronCores):
  - The two KNN-conv layers (irregular top-9 selection/gather, ~6% of FLOPs)
    run on host in fp32 with reference-exact tie-breaking.
  - The dominant FC stack runs on the 8 cores with the fc1 contraction dim
    (32768) sharded 8 ways in bf16: core i gets h2.T[F_i] and fw1.T[F_i]
    (8 MB each), computes fp32 partials for all 1024 batch rows, an
    on-device ReduceScatter leaves core i with final fc1 batch rows
    [128i:128(i+1)], then fused bias+relu and fc2 produce its 128x10 slice.
    H2D traffic is 128 MB bf16 total vs 1.15 GB for a replicated-fw1 plan
    (the host<->device link is the bottleneck at ~25-70 MB/s).
"""
import numpy as np
import ml_dtypes

import concourse.bass as bass
import concourse.tile as tile
from concourse import bacc, mybir
from concourse.bass_utils import run_bass_kernel_spmd

K, N, SCALE = 9, 8, 2
BF16 = np.dtype(ml_dtypes.bfloat16)
NCORES = 8
B = 1024
F = 32768
FSH = F // NCORES      # 4096
U = 1024
O2 = 10

_CACHE = {}


# ---------------------------------------------------------------- host conv
def _unshuffle(x, s):
    B_, C, H, W = x.shape
    return x.reshape(B_, C, H//s, s, W//s, s).transpose(0, 1, 3, 5, 2, 4).reshape(B_, C*s*s, H//s, W//s)


def _shuffle(x, s):
    B_, C, H, W = x.shape
    return x.reshape(B_, C//(s*s), s, s, H, W).transpose(0, 1, 4, 2, 5, 3).reshape(B_, C//(s*s), H*s, W*s)


def _conv_nn(x, w, b):
    x = _unshuffle(x, SCALE)
    B_, C, H, W = x.shape
    gy, gx = np.meshgrid(np.linspace(0., 1., H, dtype=np.float32),
                         np.linspace(0., 1., W, dtype=np.float32), indexing='ij')
    loc = np.broadcast_to(np.stack([gy, gx])[None], (B_, 2, H, W)).astype(np.float32)
    x = np.concatenate([x, loc], 1)
    Cf = C + 2
    xf = x.reshape(B_, Cf, H*W)
    ih = np.linspace(0, H-1, N).astype(np.int32)
    iw = np.linspace(0, W-1, N).astype(np.int32)
    samp = x[:, :, ih][:, :, :, iw].reshape(B_, Cf, N*N)
    d2 = (np.einsum('bct,bct->bt', xf, xf)[:, :, None]
          - 2.0 * np.einsum('bct,bcn->btn', xf, samp, optimize=True)
          + np.einsum('bcn,bcn->bn', samp, samp)[:, None, :]).astype(np.float32)
    # top-K nearest, ties broken toward lower candidate index (== jax top_k)
    idx = np.argsort(d2, axis=2, kind='stable')[:, :, :K]
    neigh = np.take_along_axis(samp[:, :, None, :], idx[:, None, :, :], axis=3)
    ng = neigh.transpose(0, 2, 1, 3).reshape(B_, H*W, Cf*K)
    out = ng @ w.reshape(w.shape[0], Cf*K).T.astype(np.float32) + b[None, None, :]
    out = out.transpose(0, 2, 1)
    return _shuffle(out.reshape(B_, w.shape[0], H, W), SCALE)


# ---------------------------------------------------------------- device fc
def _build_fc_kernel():
    if 'nc' in _CACHE:
        return _CACHE['nc']
    nc = bacc.Bacc("TRN2", target_bir_lowering=False, debug=False,
                   enable_asserts=False, num_devices=NCORES)
    f32 = mybir.dt.float32
    bf16 = mybir.dt.bfloat16
    h2ti = nc.dram_tensor("h2ti", (FSH, B), bf16, kind="ExternalInput").ap()
    fw1s = nc.dram_tensor("fw1s", (FSH, U), bf16, kind="ExternalInput").ap()
    fb1t = nc.dram_tensor("fb1t", (128, 8), f32, kind="ExternalInput").ap()
    fw2t = nc.dram_tensor("fw2t", (U, O2), bf16, kind="ExternalInput").ap()
    fb2r = nc.dram_tensor("fb2r", (1, O2), bf16, kind="ExternalInput").ap()
    onesr = nc.dram_tensor("onesr", (1, 128), bf16, kind="ExternalInput").ap()
    ident = nc.dram_tensor("ident", (128, 128), f32, kind="ExternalInput").ap()
    outt = nc.dram_tensor("outt", (128, O2), f32, kind="ExternalOutput").ap()

    NCH = FSH // 128       # 32 feature chunks per core

    with tile.TileContext(nc) as tc:
        with tc.tile_pool(name="wres", bufs=1) as wres, \
             tc.tile_pool(name="small", bufs=1) as spool, \
             tc.tile_pool(name="stage", bufs=2) as stpool, \
             tc.tile_pool(name="acts", bufs=1) as apool, \
             tc.tile_pool(name="ps", bufs=2, space="PSUM") as pspool, \
             tc.tile_pool(name="pst", bufs=2, space="PSUM") as ptpool, \
             tc.tile_pool(name="dram", bufs=1, space="DRAM") as dram:

            # resident weights + activations: 64KB + 64KB per partition
            wtile = wres.tile([128, NCH * U], bf16)
            htile = wres.tile([128, NCH * B], bf16)
            for c in range(NCH):
                nc.sync.dma_start(wtile[:, bass.ts(c, U)], fw1s[bass.ts(c, 128), :])
                nc.sync.dma_start(htile[:, bass.ts(c, B)], h2ti[bass.ts(c, 128), :])

            ones_t = spool.tile([1, 128], bf16)
            nc.sync.dma_start(ones_t[:], onesr[:, :])
            fb1_t = spool.tile([128, 8], f32)
            nc.sync.dma_start(fb1_t[:], fb1t[:, :])
            fb2_t = spool.tile([1, O2], bf16)
            nc.sync.dma_start(fb2_t[:], fb2r[:, :])
            id_t = spool.tile([128, 128], f32)
            nc.sync.dma_start(id_t[:], ident[:, :])
            fw2_t = spool.tile([128, 8 * O2], bf16)
            for c in range(8):
                nc.sync.dma_start(fw2_t[:, bass.ts(c, O2)], fw2t[bass.ts(c, 128), :])

            bounce_in = dram.tile([B, U], f32)
            bounce_out = dram.tile([128, U], f32)

            # fc1 partials over all 8 batch blocks
            for j in range(NCORES):
                psum = pspool.tile([128, U], f32)
                for c in range(NCH):
                    lhsT = htile[:, c * B + j * 128: c * B + (j + 1) * 128]
                    for half in range(2):
                        nc.tensor.matmul(psum[:, bass.ts(half, 512)],
                                         lhsT=lhsT,
                                         rhs=wtile[:, c * U + half * 512: c * U + (half + 1) * 512],
                                         start=(c == 0), stop=(c == NCH - 1))
                stg = stpool.tile([128, U], f32)
                nc.scalar.copy(stg[:], psum[:])
                nc.sync.dma_start(bounce_in[j * 128:(j + 1) * 128, :], stg[:])

            nc.gpsimd.collective_compute(
                "ReduceScatter", mybir.AluOpType.add,
                replica_groups=[list(range(NCORES))],
                ins=[bounce_in.opt()], outs=[bounce_out.opt()],
            )

            h1raw = apool.tile([128, U], f32)
            nc.sync.dma_start(h1raw[:], bounce_out[:])

            # transpose 128x128 blocks; relu(x + fb1) fused on the way out
            h1T = apool.tile([128, U], bf16)
            for c in range(8):
                pt = ptpool.tile([128, 128], f32)
                nc.tensor.transpose(pt[:], h1raw[:, bass.ts(c, 128)], id_t[:])
                nc.scalar.activation(h1T[:, bass.ts(c, 128)], pt[:],
                                     mybir.ActivationFunctionType.Relu,
                                     bias=fb1_t[:, c:c + 1])

            psum2 = ptpool.tile([128, O2], f32)
            for c in range(8):
                nc.tensor.matmul(psum2[:], lhsT=h1T[:, bass.ts(c, 128)],
                                 rhs=fw2_t[:, bass.ts(c, O2)],
                                 start=(c == 0), stop=False)
            nc.tensor.matmul(psum2[:], lhsT=ones_t[:], rhs=fb2_t[:],
                             start=False, stop=True)

            out_t = apool.tile([128, O2], f32)
            nc.scalar.copy(out_t[:], psum2[:])
            nc.sync.dma_start(outt[:, :], out_t[:])

    nc.compile()
    _CACHE['nc'] = nc
    return nc


def kernel(x, w1, b1, w2, b2, fw1, fb1, fw2, fb2):
    x = np.asarray(x, np.float32)
    # host: the two KNN-conv layers (exact fp32 ranking, reference tie-break)
    h1 = np.maximum(_conv_nn(x, np.asarray(w1, np.float32), np.asarray(b1, np.float32)), 0)
    h2 = np.maximum(_conv_nn(h1, np.asarray(w2, np.float32), np.asarray(b2, np.float32)), 0)
    h2 = h2.reshape(B, -1)                              # (1024, 32768)

    nc = _build_fc_kernel()
    fw1 = np.asarray(fw1, np.float32)
    h2t = h2.T                                          # (32768, 1024) view
    fw1t = fw1.T                                        # (32768, 1024) view
    fb1t = np.ascontiguousarray(np.asarray(fb1, np.float32).reshape(8, 128).T)
    fw2t = np.asarray(fw2, np.float32).T.astype(BF16)
    fb2r = np.asarray(fb2, np.float32).reshape(1, O2).astype(BF16)
    onesr = np.ones((1, 128), BF16)
    ident = np.eye(128, dtype=np.float32)
    in_maps = []
    for i in range(NCORES):
        sl = slice(i * FSH, (i + 1) * FSH)
        in_maps.append(dict(h2ti=h2t[sl].astype(BF16),
                            fw1s=fw1t[sl].astype(BF16),
                            fb1t=fb1t, fw2t=fw2t, fb2r=fb2r,
                            onesr=onesr, ident=ident))
    res = run_bass_kernel_spmd(nc, in_maps, core_ids=list(range(NCORES)))
    out = np.empty((B, O2), np.float32)
    for i in range(NCORES):
        out[i * 128:(i + 1) * 128] = res.results[i]["outt"]
    return out
